# revision 12
# baseline (speedup 1.0000x reference)
"""Trainium2 Bass kernel for the EnhancedNavigationLTC model (v2, PE-centric).

Pure data parallel over batch: 512 rows sharded 8 ways (64 per core); all
parameters replicated.  Per core:

  phase 1 (parallel over B*T): sensory reduction.  Per 16-step chunk, 64
      ACT sigmoids (scale/bias folded per unit) in fp16 feed accumulating
      PE matmuls whose stationary rows are (nd, v)-major, producing
      snd[(nd, v), (t, b)] written to DRAM scratch in fp16.

  phase 2 (sequential, T steps x 4 unfolds): everything PE-centric in
      "transposed" space (units on partitions, batch in free):
        argT_j[(p2, v), b] = esig[u,v]*vst[b,u] - esigmu[u,v],  u = 2j+p2,
      built by 32 selector matmuls (stationary S_arg[j] [65,128], moving
      vstT [65,64] fp16); sigmoid runs on ACT (PSUM->SBUF fp16); the two
      weighted reductions over u are 32 accumulating matmuls (stationary
      S_red[j] [128,128] = wp / wp*erev diagonal-in-v) into a single
      out_red[(nd, v), b] PSUM tile.  The ODE update runs on [64,64] tiles
      where cm_t/gleak are per-partition scalars; the state feeds back with
      one fp16 copy (no transposes anywhere in the loop).

  head: output affine + LayerNorm + FC, on-device (as baseline).
"""

import numpy as np

U = 64
I = 128
O = 15
UNFOLDS = 4
EPS = 1e-8
LN_EPS = 1e-5
B_FULL, T_FULL = 512, 512
N_CORES = 8
B_CORE = B_FULL // N_CORES          # 64
TS_CHUNK = 16                       # timesteps per phase-1 chunk
CHUNK_COLS = TS_CHUNK * B_CORE      # 1024


def _softplus(x):
    return np.log1p(np.exp(-np.abs(x))) + np.maximum(x, 0.0)


def _host_consts(input_w, input_b, sensory_w, sensory_mu, sensory_sigma,
                 sensory_erev, w, mu, sigma, erev, gleak, vleak, cm,
                 output_w, output_b, ln_w, ln_b, fc_w, fc_b):
    """All parameter-derived device constants as numpy arrays."""
    f32, f16 = np.float32, np.float16
    wp = _softplus(np.asarray(w, f32))                  # (U, U)  [u, v]
    swp = _softplus(np.asarray(sensory_w, f32))         # (I, U)
    gleak_p = _softplus(np.asarray(gleak, f32))         # (U,)
    cm_t = (_softplus(np.asarray(cm, f32)) * UNFOLDS)   # (U,)

    # ---- phase 1 (sensory) ----
    se = np.asarray(sensory_erev, f32)                  # +-1  (I, U)
    ss = np.asarray(sensory_sigma, f32)
    iw = np.asarray(input_w, f32)
    ib = np.asarray(input_b, f32)
    smu = np.asarray(sensory_mu, f32)
    sens_scale = (se * ss * iw[:, None]).astype(f32)                # (I, U)
    sens_bias = (se * ss * (ib[:, None] - smu)).astype(f32)         # (I, U)
    Ks = (swp * (se < 0)).sum(axis=0).astype(f32)       # (U,)

    # stationary per u: [I, 128] with col m=(nd,v): nd=0 -> swp, nd=1 -> swp*se
    sens_lhsT = np.zeros((I, U, 2 * U), f32)
    for u in range(U):
        sens_lhsT[:, u, u] = swp[:, u]
        sens_lhsT[:, u, U + u] = swp[:, u] * se[:, u]
    sens_lhsT = sens_lhsT.reshape(I, U * 2 * U).astype(f16)   # (128, 8192)

    # ---- phase 2 (recurrent) ----
    ee = np.asarray(erev, f32)                          # +-1  (U, U) [u, v]
    esig = ee * np.asarray(sigma, f32)
    esigmu = esig * np.asarray(mu, f32)
    K = (wp * (ee < 0)).sum(axis=0).astype(f32)         # (V,)

    # S_arg[j]: [65, 128], col m = (p2, v): argT = S_arg[j].T @ [vstT; ones]
    S_arg = np.zeros((32, U + 1, 128), f32)
    for j in range(32):
        for p2 in range(2):
            u = 2 * j + p2
            S_arg[j, u, 64 * p2:64 * p2 + 64] = esig[u, :]
            S_arg[j, U, 64 * p2:64 * p2 + 64] = -esigmu[u, :]
    S_arg = S_arg.transpose(1, 0, 2).reshape(U + 1, 32 * 128).astype(f16)

    # S_red[j]: [128, 128], row k=(p2, v'), col m=(nd, v): diag in v
    S_red = np.zeros((32, 128, 128), f32)
    for j in range(32):
        for p2 in range(2):
            u = 2 * j + p2
            for v in range(U):
                S_red[j, 64 * p2 + v, v] = wp[u, v]
                S_red[j, 64 * p2 + v, U + v] = wp[u, v] * ee[u, v]
    S_red = S_red.transpose(1, 0, 2).reshape(128, 32 * 128).astype(f16)

    glvl = gleak_p * np.asarray(vleak, f32)
    CN = -K + glvl - Ks                                 # add to num rows
    CD = cm_t + gleak_p + K + Ks + EPS                  # add to den rows
    CNCD_T = np.concatenate([CN, CD])[:, None].astype(f32)   # [128, 1]
    cmt_T = cm_t[:, None].astype(f32)                        # [64, 1]

    # ---- head ----
    tile_b = lambda vec: np.tile(np.asarray(vec, f32)[None, :], (B_CORE, 1))
    return {
        "sens_scale": sens_scale, "sens_bias": sens_bias,
        "sens_lhsT": sens_lhsT,
        "S_arg": S_arg, "S_red": S_red,
        "CNCD_T": CNCD_T, "cmt_T": cmt_T,
        "ident": np.eye(64, dtype=f16),
        "identf": np.eye(64, dtype=f32),
        "ow_l": tile_b(output_w), "ob_l": tile_b(output_b),
        "lng_l": tile_b(ln_w), "lnb_l": tile_b(ln_b),
        "fcwT": np.asarray(fc_w, f32).T.copy().astype(f32),  # (U, O)
        "fcb_l": tile_b(fc_b),                               # (B_CORE, O)
    }


def _build_program(T, repeats=1, unroll=False):
    """Build the per-core SPMD Bass program for T timesteps.

    repeats>1 re-runs the whole computation (timing by differencing).
    unroll=True replaces For_i hardware loops with Python unrolling (for
    TimelineSim, which cannot resolve register-mode branches).
    """
    import concourse.bass as bass
    import concourse.tile as tile
    from concourse import bacc, mybir
    from contextlib import ExitStack

    F32 = mybir.dt.float32
    F16 = mybir.dt.float16
    AF = mybir.ActivationFunctionType
    ALU = mybir.AluOpType
    AX = mybir.AxisListType

    n_chunks = T // TS_CHUNK
    nc = bacc.Bacc("TRN2", target_bir_lowering=False, debug=False)

    d_xT = nc.dram_tensor("xT", [I, T * B_CORE], F16, kind="ExternalInput")
    cshape = {
        "sens_scale": ([I, U], F32), "sens_bias": ([I, U], F32),
        "sens_lhsT": ([I, U * 2 * U], F16),
        "S_arg": ([U + 1, 32 * 128], F16), "S_red": ([128, 32 * 128], F16),
        "CNCD_T": ([128, 1], F32), "cmt_T": ([U, 1], F32),
        "ident": ([U, U], F16), "identf": ([U, U], F32),
        "ow_l": ([B_CORE, U], F32), "ob_l": ([B_CORE, U], F32),
        "lng_l": ([B_CORE, U], F32), "lnb_l": ([B_CORE, U], F32),
        "fcwT": ([U, O], F32), "fcb_l": ([B_CORE, O], F32),
    }
    d_c = {k: nc.dram_tensor(k, shp, dt, kind="ExternalInput")
           for k, (shp, dt) in cshape.items()}
    d_out = nc.dram_tensor("out", [B_CORE, O], F32, kind="ExternalOutput")
    # scratch [128=(nd,v), (T+2)*64] (padded for prefetch overrun)
    d_scr = nc.dram_tensor("scr", [128, (T + 2) * B_CORE], F16)

    with tile.TileContext(nc) as tc:
        with ExitStack() as ctx:
            cpool = ctx.enter_context(tc.tile_pool(name="consts", bufs=1))
            c = {}
            for k, (shp, dt) in cshape.items():
                c[k] = cpool.tile(shp, dt, tag=k, name=k)
                nc.sync.dma_start(c[k][:], d_c[k][:])

            def loop_i(lo, hi, step, body):
                if unroll:
                    for i in range(lo, hi, step):
                        body(i)
                else:
                    with tc.For_i(lo, hi, step) as iv:
                        body(iv)

            for _rep in range(repeats):
                # ============ phase 1 ============
                with ExitStack() as p1:
                    pool1 = p1.enter_context(tc.tile_pool(name="p1", bufs=2))
                    sgp = p1.enter_context(tc.tile_pool(name="p1sg", bufs=3))
                    pps1 = p1.enter_context(
                        tc.tile_pool(name="p1ps", bufs=2, space="PSUM"))

                    def p1_body(iv):
                        xc = pool1.tile([I, CHUNK_COLS], F16, tag="xc")
                        nc.sync.dma_start(xc[:], d_xT[:, bass.ds(iv, CHUNK_COLS)])
                        ps_nd = pps1.tile([128, CHUNK_COLS], F32, tag="psnd")
                        for u in range(U):
                            sg = sgp.tile([I, CHUNK_COLS], F16, tag="sg")
                            nc.scalar.activation(
                                sg[:], xc[:], AF.Sigmoid,
                                bias=c["sens_bias"][:, u:u + 1],
                                scale=c["sens_scale"][:, u:u + 1])
                            for n in range(CHUNK_COLS // 512):
                                nc.tensor.matmul(
                                    ps_nd[:, 512 * n:512 * n + 512],
                                    c["sens_lhsT"][:, 128 * u:128 * u + 128],
                                    sg[:, 512 * n:512 * n + 512],
                                    start=(u == 0), stop=(u == U - 1))
                        ev = pool1.tile([128, CHUNK_COLS], F16, tag="ev")
                        nc.vector.tensor_copy(ev[:], ps_nd[:])
                        nc.sync.dma_start(
                            d_scr[:, bass.ds(iv, CHUNK_COLS)], ev[:])

                    loop_i(0, n_chunks * CHUNK_COLS, CHUNK_COLS, p1_body)

                # =================== phase 2 ===================
                with ExitStack() as pst:
                  spool = pst.enter_context(tc.tile_pool(name="p2state", bufs=1))
                  vstT = spool.tile([U + 1, B_CORE], F16, tag="vstT")
                  nc.vector.memset(vstT[0:U, :], 0.0)
                  nc.vector.memset(vstT[U:U + 1, :], 1.0)

                  with ExitStack() as p2:
                    pool2 = p2.enter_context(tc.tile_pool(name="p2", bufs=2))
                    sndp = p2.enter_context(tc.tile_pool(name="p2snd", bufs=3))
                    sigp = p2.enter_context(tc.tile_pool(name="p2sig", bufs=3))
                    ppsa = p2.enter_context(
                        tc.tile_pool(name="p2psa", bufs=3, space="PSUM"))
                    ppsr = p2.enter_context(
                        tc.tile_pool(name="p2psr", bufs=2, space="PSUM"))

                    def p2_body(iv):
                        snd_t = sndp.tile([128, B_CORE], F16, tag="snd")
                        nc.sync.dma_start(
                            snd_t[:], d_scr[:, bass.ds(iv, B_CORE)])
                        base = pool2.tile([128, B_CORE], F32, tag="base")
                        nc.gpsimd.tensor_scalar(
                            base[:], snd_t[:], c["CNCD_T"][:, 0:1], None,
                            op0=ALU.add)

                        for uf in range(UNFOLDS):
                            out_red = ppsr.tile([128, B_CORE], F32, tag="ored")
                            cmtv = pool2.tile([U, B_CORE], F32, tag="cmtv")
                            nc.gpsimd.tensor_scalar(
                                cmtv[:], vstT[0:U, :], c["cmt_T"][:, 0:1],
                                None, op0=ALU.mult)
                            for ch in range(2):      # 2 chunks of 16 j each
                                psT = ppsa.tile([128, 16 * B_CORE], F32,
                                                tag="psT")
                                for jj in range(16):
                                    j = 16 * ch + jj
                                    nc.tensor.matmul(
                                        psT[:, 64 * jj:64 * jj + 64],
                                        c["S_arg"][:, 128 * j:128 * j + 128],
                                        vstT[:], start=True, stop=True)
                                sig = sigp.tile([128, 16 * B_CORE], F16,
                                                tag="sig")
                                nc.scalar.activation(sig[:], psT[:],
                                                     AF.Sigmoid)
                                for jj in range(16):
                                    j = 16 * ch + jj
                                    nc.tensor.matmul(
                                        out_red[:],
                                        c["S_red"][:, 128 * j:128 * j + 128],
                                        sig[:, 64 * jj:64 * jj + 64],
                                        start=(j == 0), stop=(j == 31))
                            tot = pool2.tile([128, B_CORE], F32, tag="tot")
                            nc.vector.tensor_add(tot[:], out_red[:], base[:])
                            rec = pool2.tile([U, B_CORE], F32, tag="rec")
                            nc.vector.reciprocal(rec[:], tot[U:2 * U, :])
                            num2 = pool2.tile([U, B_CORE], F32, tag="num2")
                            nc.vector.tensor_add(num2[:], tot[0:U, :], cmtv[:])
                            v_new = pool2.tile([U, B_CORE], F32, tag="vnew")
                            nc.vector.tensor_mul(v_new[:], num2[:], rec[:])
                            nc.gpsimd.tensor_copy(vstT[0:U, :], v_new[:])

                    loop_i(0, T * B_CORE, B_CORE, p2_body)

                  # =================== head ===================
                  with ExitStack() as ph:
                    poolh = ph.enter_context(tc.tile_pool(name="ph", bufs=1))
                    ppsh = ph.enter_context(
                        tc.tile_pool(name="phps", bufs=1, space="PSUM"))
                    vst32 = poolh.tile([U, B_CORE], F32, tag="vst32")
                    nc.scalar.copy(vst32[:], vstT[0:U, :])
                    hp = ppsh.tile([B_CORE, U], F32, tag="hp")
                    nc.tensor.transpose(hp[:], vst32[:], c["identf"][:])
                    h = poolh.tile([B_CORE, U], F32, tag="h")
                    nc.vector.tensor_mul(h[:], hp[:], c["ow_l"][:])
                    nc.vector.tensor_add(h[:], h[:], c["ob_l"][:])
                    mean = poolh.tile([B_CORE, 1], F32, tag="mean")
                    nc.vector.tensor_reduce(mean[:], h[:], axis=AX.X, op=ALU.add)
                    nc.vector.tensor_scalar_mul(mean[:], mean[:], 1.0 / U)
                    xc2 = poolh.tile([B_CORE, U], F32, tag="xc2")
                    nc.vector.tensor_scalar(
                        xc2[:], h[:], mean[:], None, op0=ALU.subtract)
                    sq = poolh.tile([B_CORE, U], F32, tag="sq")
                    nc.vector.tensor_mul(sq[:], xc2[:], xc2[:])
                    var = poolh.tile([B_CORE, 1], F32, tag="var")
                    nc.vector.tensor_reduce(var[:], sq[:], axis=AX.X, op=ALU.add)
                    nc.vector.tensor_scalar(
                        var[:], var[:], 1.0 / U, LN_EPS, op0=ALU.mult, op1=ALU.add)
                    sd = poolh.tile([B_CORE, 1], F32, tag="sd")
                    nc.scalar.sqrt(sd[:], var[:])
                    rstd = poolh.tile([B_CORE, 1], F32, tag="rstd")
                    nc.vector.reciprocal(rstd[:], sd[:])
                    nc.vector.tensor_scalar(
                        xc2[:], xc2[:], rstd[:], None, op0=ALU.mult)
                    nc.vector.tensor_mul(xc2[:], xc2[:], c["lng_l"][:])
                    nc.vector.tensor_add(xc2[:], xc2[:], c["lnb_l"][:])
                    hTp = ppsh.tile([U, B_CORE], F32, tag="hTp")
                    nc.tensor.transpose(hTp[:], xc2[:], c["identf"][:])
                    hT = poolh.tile([U, B_CORE], F32, tag="hT")
                    nc.scalar.copy(hT[:], hTp[:])
                    ps_fc = ppsh.tile([B_CORE, O], F32, tag="psfc")
                    nc.tensor.matmul(ps_fc[:], hT[:], c["fcwT"][:],
                                     start=True, stop=True)
                    res = poolh.tile([B_CORE, O], F32, tag="res")
                    nc.vector.tensor_add(res[:], ps_fc[:], c["fcb_l"][:])
                    nc.sync.dma_start(d_out[:], res[:])

    nc.finalize()
    return nc


def _make_xT(x_core, T):
    """(B_CORE, T, I) -> fp16 [I, T*B_CORE] with col = t*64 + b."""
    xt = np.ascontiguousarray(x_core.transpose(2, 1, 0))  # (I, T, B)
    return xt.reshape(I, T * B_CORE).astype(np.float16)


_PROGRAM_CACHE = {}


def kernel(**inputs):
    import sys
    if '/opt/trn_rl_repo' not in sys.path:
        sys.path.insert(0, '/opt/trn_rl_repo')
    from concourse.bass_utils import run_bass_kernel_spmd

    x = np.asarray(inputs["x"], np.float32)
    B, T = x.shape[0], x.shape[1]
    consts = _host_consts(**{k: np.asarray(v) for k, v in inputs.items()
                             if k != "x"})

    if T not in _PROGRAM_CACHE:
        _PROGRAM_CACHE[T] = _build_program(T)
    nc = _PROGRAM_CACHE[T]

    in_maps = []
    for g in range(N_CORES):
        m = dict(consts)
        m["xT"] = _make_xT(x[g * B_CORE:(g + 1) * B_CORE], T)
        in_maps.append(m)
    res = run_bass_kernel_spmd(nc, in_maps, list(range(N_CORES)))
    return np.concatenate([res.results[g]["out"] for g in range(N_CORES)],
                          axis=0)


# revision 32
# speedup vs baseline: 6.7737x; 6.7737x over previous
"""Trainium2 Bass kernel for the EnhancedNavigationLTC model (v2, PE-centric).

Pure data parallel over batch: 512 rows sharded 8 ways (64 per core); all
parameters replicated.  Per core:

  phase 1 (parallel over B*T): sensory reduction.  Per 16-step chunk, 64
      ACT sigmoids (scale/bias folded per unit) in fp16 feed accumulating
      PE matmuls whose stationary rows are (nd, v)-major, producing
      snd[(nd, v), (t, b)] written to DRAM scratch in fp16.

  phase 2 (sequential, T steps x 4 unfolds): everything PE-centric in
      "transposed" space (units on partitions, batch in free):
        argT_j[(p2, v), b] = esig[u,v]*vst[b,u] - esigmu[u,v],  u = 2j+p2,
      built by 32 selector matmuls (stationary S_arg[j] [65,128], moving
      vstT [65,64] fp16); sigmoid runs on ACT (PSUM->SBUF fp16); the two
      weighted reductions over u are 32 accumulating matmuls (stationary
      S_red[j] [128,128] = wp / wp*erev diagonal-in-v) into a single
      out_red[(nd, v), b] PSUM tile.  The ODE update runs on [64,64] tiles
      where cm_t/gleak are per-partition scalars; the state feeds back with
      one fp16 copy (no transposes anywhere in the loop).

  head: output affine + LayerNorm + FC, on-device (as baseline).
"""

import numpy as np

U = 64
I = 128
O = 15
UNFOLDS = 4
EPS = 1e-8
LN_EPS = 1e-5
B_FULL, T_FULL = 512, 512
N_CORES = 8
B_CORE = B_FULL // N_CORES          # 64
TS_CHUNK = 16                       # timesteps per phase-1 chunk
CHUNK_COLS = TS_CHUNK * B_CORE      # 1024


def _softplus(x):
    return np.log1p(np.exp(-np.abs(x))) + np.maximum(x, 0.0)


def _host_consts(input_w, input_b, sensory_w, sensory_mu, sensory_sigma,
                 sensory_erev, w, mu, sigma, erev, gleak, vleak, cm,
                 output_w, output_b, ln_w, ln_b, fc_w, fc_b):
    """All parameter-derived device constants as numpy arrays."""
    f32, f16 = np.float32, np.float16
    wp = _softplus(np.asarray(w, f32))                  # (U, U)  [u, v]
    swp = _softplus(np.asarray(sensory_w, f32))         # (I, U)
    gleak_p = _softplus(np.asarray(gleak, f32))         # (U,)
    cm_t = (_softplus(np.asarray(cm, f32)) * UNFOLDS)   # (U,)

    # ---- phase 1 (sensory) ----
    se = np.asarray(sensory_erev, f32)                  # +-1  (I, U)
    ss = np.asarray(sensory_sigma, f32)
    iw = np.asarray(input_w, f32)
    ib = np.asarray(input_b, f32)
    smu = np.asarray(sensory_mu, f32)
    sens_scale = (se * ss * iw[:, None]).astype(f32)                # (I, U)
    sens_bias = (se * ss * (ib[:, None] - smu)).astype(f32)         # (I, U)
    Ks = (swp * (se < 0)).sum(axis=0).astype(f32)       # (U,)

    # stationary per u: [I, 128] with col m=(nd,v): nd=0 -> swp, nd=1 -> swp*se
    sens_lhsT = np.zeros((I, U, 2 * U), f32)
    for u in range(U):
        sens_lhsT[:, u, u] = swp[:, u]
        sens_lhsT[:, u, U + u] = swp[:, u] * se[:, u]
    sens_lhsT = sens_lhsT.reshape(I, U * 2 * U).astype(f16)   # (128, 8192)

    # ---- phase 2 (recurrent) ----
    ee = np.asarray(erev, f32)                          # +-1  (U, U) [u, v]
    esig = ee * np.asarray(sigma, f32)
    esigmu = esig * np.asarray(mu, f32)
    K = (wp * (ee < 0)).sum(axis=0).astype(f32)         # (V,)

    # S_arg[j]: [65, 128], col m = (p2, v): argT = S_arg[j].T @ [vstT; ones]
    S_arg = np.zeros((32, U + 1, 128), f32)
    for j in range(32):
        for p2 in range(2):
            u = 2 * j + p2
            S_arg[j, u, 64 * p2:64 * p2 + 64] = esig[u, :]
            S_arg[j, U, 64 * p2:64 * p2 + 64] = -esigmu[u, :]
    S_arg = S_arg.transpose(1, 0, 2).reshape(U + 1, 32 * 128).astype(f16)

    # S_red[j]: [128, 128], row k=(p2, v'), col m=(nd, v): diag in v
    S_red = np.zeros((32, 128, 128), f32)
    for j in range(32):
        for p2 in range(2):
            u = 2 * j + p2
            for v in range(U):
                S_red[j, 64 * p2 + v, v] = wp[u, v]
                S_red[j, 64 * p2 + v, U + v] = wp[u, v] * ee[u, v]
    S_red = S_red.transpose(1, 0, 2).reshape(128, 32 * 128).astype(f16)

    glvl = gleak_p * np.asarray(vleak, f32)
    CN = -K + glvl - Ks                                 # add to num rows
    CD = cm_t + gleak_p + K + Ks + EPS                  # add to den rows
    CNCD_T = np.concatenate([CN, CD])[:, None].astype(f32)   # [128, 1]
    cmt_T = cm_t[:, None].astype(f32)                        # [64, 1]

    # ---- head ----
    tile_b = lambda vec: np.tile(np.asarray(vec, f32)[None, :], (B_CORE, 1))
    return {
        "sens_scale": sens_scale, "sens_bias": sens_bias,
        "sens_lhsT": sens_lhsT,
        "S_arg": S_arg, "S_red": S_red,
        "CNCD_T": CNCD_T, "cmt_T": cmt_T,
        "ident": np.eye(64, dtype=f16),
        "identf": np.eye(64, dtype=f32),
        "ow_l": tile_b(output_w), "ob_l": tile_b(output_b),
        "lng_l": tile_b(ln_w), "lnb_l": tile_b(ln_b),
        "fcwT": np.asarray(fc_w, f32).T.copy().astype(f32),  # (U, O)
        "fcb_l": tile_b(fc_b),                               # (B_CORE, O)
    }


def _emit_head(nc, tc, c, poolh, ppsh, vstT, d_out, mybir):
    F32 = mybir.dt.float32
    ALU = mybir.AluOpType
    AX = mybir.AxisListType
    vst32 = poolh.tile([U, B_CORE], F32, tag="vst32")
    nc.scalar.copy(vst32[:], vstT[0:U, :])
    hp = ppsh.tile([B_CORE, U], F32, tag="hp")
    nc.tensor.transpose(hp[:], vst32[:], c["identf"][:])
    h = poolh.tile([B_CORE, U], F32, tag="h")
    nc.vector.tensor_mul(h[:], hp[:], c["ow_l"][:])
    nc.vector.tensor_add(h[:], h[:], c["ob_l"][:])
    mean = poolh.tile([B_CORE, 1], F32, tag="mean")
    nc.vector.tensor_reduce(mean[:], h[:], axis=AX.X, op=ALU.add)
    nc.vector.tensor_scalar_mul(mean[:], mean[:], 1.0 / U)
    xc2 = poolh.tile([B_CORE, U], F32, tag="xc2")
    nc.vector.tensor_scalar(xc2[:], h[:], mean[:], None, op0=ALU.subtract)
    sq = poolh.tile([B_CORE, U], F32, tag="sq")
    nc.vector.tensor_mul(sq[:], xc2[:], xc2[:])
    var = poolh.tile([B_CORE, 1], F32, tag="var")
    nc.vector.tensor_reduce(var[:], sq[:], axis=AX.X, op=ALU.add)
    nc.vector.tensor_scalar(
        var[:], var[:], 1.0 / U, LN_EPS, op0=ALU.mult, op1=ALU.add)
    sd = poolh.tile([B_CORE, 1], F32, tag="sd")
    nc.scalar.sqrt(sd[:], var[:])
    rstd = poolh.tile([B_CORE, 1], F32, tag="rstd")
    nc.vector.reciprocal(rstd[:], sd[:])
    nc.vector.tensor_scalar(xc2[:], xc2[:], rstd[:], None, op0=ALU.mult)
    nc.vector.tensor_mul(xc2[:], xc2[:], c["lng_l"][:])
    nc.vector.tensor_add(xc2[:], xc2[:], c["lnb_l"][:])
    hTp = ppsh.tile([U, B_CORE], F32, tag="hTp")
    nc.tensor.transpose(hTp[:], xc2[:], c["identf"][:])
    hT = poolh.tile([U, B_CORE], F32, tag="hT")
    nc.scalar.copy(hT[:], hTp[:])
    ps_fc = ppsh.tile([B_CORE, O], F32, tag="psfc")
    nc.tensor.matmul(ps_fc[:], hT[:], c["fcwT"][:], start=True, stop=True)
    res = poolh.tile([B_CORE, O], F32, tag="res")
    nc.vector.tensor_add(res[:], ps_fc[:], c["fcb_l"][:])
    nc.sync.dma_start(d_out[:], res[:])


def _build_program(T, repeats=1, unroll=False, rep_p1=1, rep_p2=1,
                   ablate=(), interleave=False):
    """Build the per-core SPMD Bass program for T timesteps.

    repeats>1 re-runs the whole computation (timing by differencing).
    unroll=True replaces For_i hardware loops with Python unrolling (for
    TimelineSim, which cannot resolve register-mode branches).
    """
    import concourse.bass as bass
    import concourse.tile as tile
    from concourse import bacc, mybir
    from contextlib import ExitStack

    F32 = mybir.dt.float32
    F16 = mybir.dt.float16
    AF = mybir.ActivationFunctionType
    ALU = mybir.AluOpType
    AX = mybir.AxisListType

    n_chunks = T // TS_CHUNK
    nc = bacc.Bacc("TRN2", target_bir_lowering=False, debug=False)

    d_xT = nc.dram_tensor("xT", [I, T * B_CORE], F16, kind="ExternalInput")
    cshape = {
        "sens_scale": ([I, U], F32), "sens_bias": ([I, U], F32),
        "sens_lhsT": ([I, U * 2 * U], F16),
        "S_arg": ([U + 1, 32 * 128], F16), "S_red": ([128, 32 * 128], F16),
        "CNCD_T": ([128, 1], F32), "cmt_T": ([U, 1], F32),
        "ident": ([U, U], F16), "identf": ([U, U], F32),
        "ow_l": ([B_CORE, U], F32), "ob_l": ([B_CORE, U], F32),
        "lng_l": ([B_CORE, U], F32), "lnb_l": ([B_CORE, U], F32),
        "fcwT": ([U, O], F32), "fcb_l": ([B_CORE, O], F32),
    }
    d_c = {k: nc.dram_tensor(k, shp, dt, kind="ExternalInput")
           for k, (shp, dt) in cshape.items()}
    d_out = nc.dram_tensor("out", [B_CORE, O], F32, kind="ExternalOutput")
    # scratch [128=(nd,v), (T+2)*64] (padded for prefetch overrun)
    d_scr = nc.dram_tensor("scr", [128, (T + 2) * B_CORE], F16)

    with tile.TileContext(nc) as tc:
        with ExitStack() as ctx:
            cpool = ctx.enter_context(tc.tile_pool(name="consts", bufs=1))
            c = {}
            for k, (shp, dt) in cshape.items():
                c[k] = cpool.tile(shp, dt, tag=k, name=k)
                nc.sync.dma_start(c[k][:], d_c[k][:])

            def loop_i(lo, hi, step, body):
                if unroll:
                    for i in range(lo, hi, step):
                        body(i)
                else:
                    with tc.For_i(lo, hi, step) as iv:
                        body(iv)

            for _rep in range(repeats):
              if interleave:
                with ExitStack() as pst_i:
                  spool = pst_i.enter_context(
                      tc.tile_pool(name="p2state", bufs=1))
                  with ExitStack() as pall:
                    pool1 = pall.enter_context(tc.tile_pool(name="p1", bufs=2))
                    sgp = pall.enter_context(tc.tile_pool(name="p1sg", bufs=3))
                    pps1 = pall.enter_context(
                        tc.tile_pool(name="p1ps", bufs=1, space="PSUM"))
                    pool2 = pall.enter_context(tc.tile_pool(name="p2", bufs=2))
                    sndp = pall.enter_context(
                        tc.tile_pool(name="p2snd", bufs=3))
                    sigp = pall.enter_context(
                        tc.tile_pool(name="p2sig", bufs=3))
                    ppsa = pall.enter_context(
                        tc.tile_pool(name="p2psa", bufs=3, space="PSUM"))
                    ppsr = pall.enter_context(
                        tc.tile_pool(name="p2psr", bufs=2, space="PSUM"))

                    vstT = spool.tile([U + 1, B_CORE], F16, tag="vstT")
                    nc.vector.memset(vstT[0:U, :], 0.0)
                    nc.vector.memset(vstT[U:U + 1, :], 1.0)

                    def p1_sig_mm(xc, ps_nd, u):
                        sg = sgp.tile([I, CHUNK_COLS], F16, tag="sg")
                        nc.scalar.activation(
                            sg[:], xc[:], AF.Sigmoid,
                            bias=c["sens_bias"][:, u:u + 1],
                            scale=c["sens_scale"][:, u:u + 1])
                        for n in range(CHUNK_COLS // 512):
                            nc.tensor.matmul(
                                ps_nd[:, 512 * n:512 * n + 512],
                                c["sens_lhsT"][:, 128 * u:128 * u + 128],
                                sg[:, 512 * n:512 * n + 512],
                                start=(u == 0), stop=(u == U - 1))

                    def p1_flush(ps_nd, col0):
                        ev = pool1.tile([128, CHUNK_COLS], F16, tag="ev")
                        nc.vector.tensor_copy(ev[:], ps_nd[:])
                        nc.sync.dma_start(
                            d_scr[:, bass.ds(col0, CHUNK_COLS)], ev[:])

                    def p2_step(iv, s):
                        snd_t = sndp.tile([128, B_CORE], F16, tag="snd")
                        nc.sync.dma_start(
                            snd_t[:],
                            d_scr[:, bass.ds(iv + B_CORE * s, B_CORE)])
                        base = pool2.tile([128, B_CORE], F32, tag="base")
                        nc.gpsimd.tensor_scalar(
                            base[:], snd_t[:], c["CNCD_T"][:, 0:1], None,
                            op0=ALU.add)
                        for uf in range(UNFOLDS):
                            out_red = ppsr.tile([128, B_CORE], F32,
                                                tag="ored")
                            cmtv = pool2.tile([U, B_CORE], F32, tag="cmtv")
                            nc.gpsimd.tensor_scalar(
                                cmtv[:], vstT[0:U, :], c["cmt_T"][:, 0:1],
                                None, op0=ALU.mult)
                            for ch in range(4):
                                psT = ppsa.tile([128, 8 * B_CORE], F32,
                                                tag="psT")
                                for jj in range(8):
                                    j = 8 * ch + jj
                                    nc.tensor.matmul(
                                        psT[:, 64 * jj:64 * jj + 64],
                                        c["S_arg"][:, 128 * j:128 * j + 128],
                                        vstT[:], start=True, stop=True)
                                sig = sigp.tile([128, 8 * B_CORE], F16,
                                                tag="sig")
                                nc.scalar.activation(sig[:], psT[:],
                                                     AF.Sigmoid)
                                for jj in range(8):
                                    j = 8 * ch + jj
                                    nc.tensor.matmul(
                                        out_red[:],
                                        c["S_red"][:, 128 * j:128 * j + 128],
                                        sig[:, 64 * jj:64 * jj + 64],
                                        start=(j == 0), stop=(j == 31))
                            tot = pool2.tile([128, B_CORE], F32, tag="tot")
                            nc.vector.tensor_add(tot[:], out_red[:], base[:])
                            rec = pool2.tile([U, B_CORE], F32, tag="rec")
                            nc.vector.reciprocal(rec[:], tot[U:2 * U, :])
                            num2 = pool2.tile([U, B_CORE], F32, tag="num2")
                            nc.vector.tensor_add(num2[:], tot[0:U, :],
                                                 cmtv[:])
                            v_new = pool2.tile([U, B_CORE], F32, tag="vnew")
                            nc.vector.tensor_mul(v_new[:], num2[:], rec[:])
                            nc.gpsimd.tensor_copy(vstT[0:U, :], v_new[:])

                    # prologue: full phase-1 chunk 0
                    xc0 = pool1.tile([I, CHUNK_COLS], F16, tag="xc")
                    nc.sync.dma_start(xc0[:], d_xT[:, 0:CHUNK_COLS])
                    ps0 = pps1.tile([128, CHUNK_COLS], F32, tag="psnd")
                    for u in range(U):
                        p1_sig_mm(xc0, ps0, u)
                    p1_flush(ps0, 0)

                    def pair_body(iv, last=False):
                        if not last:
                            xc = pool1.tile([I, CHUNK_COLS], F16, tag="xc")
                            nc.sync.dma_start(
                                xc[:],
                                d_xT[:, bass.ds(iv + CHUNK_COLS,
                                                CHUNK_COLS)])
                            ps_nd = pps1.tile([128, CHUNK_COLS], F32,
                                              tag="psnd")
                        for s in range(TS_CHUNK):
                            p2_step(iv, s)
                            if not last:
                                for q in range(4):
                                    p1_sig_mm(xc, ps_nd, 4 * s + q)
                        if not last:
                            p1_flush(ps_nd, iv + CHUNK_COLS)

                    loop_i(0, (n_chunks - 1) * CHUNK_COLS, CHUNK_COLS,
                           pair_body)
                    pair_body((n_chunks - 1) * CHUNK_COLS, last=True)

                  # ---- head (p1/p2 pools released) ----
                  with ExitStack() as ph:
                    poolh = ph.enter_context(
                        tc.tile_pool(name="ph", bufs=1))
                    ppsh = ph.enter_context(
                        tc.tile_pool(name="phps", bufs=1, space="PSUM"))
                    _emit_head(nc, tc, c, poolh, ppsh, vstT, d_out,
                               mybir)
                continue
              for _repp1 in range(rep_p1):
                # ============ phase 1 ============
                with ExitStack() as p1:
                    pool1 = p1.enter_context(tc.tile_pool(name="p1", bufs=2))
                    sgp = p1.enter_context(tc.tile_pool(name="p1sg", bufs=3))
                    pps1 = p1.enter_context(
                        tc.tile_pool(name="p1ps", bufs=2, space="PSUM"))

                    def p1_body(iv):
                        xc = pool1.tile([I, CHUNK_COLS], F16, tag="xc")
                        nc.sync.dma_start(xc[:], d_xT[:, bass.ds(iv, CHUNK_COLS)])
                        ps_nd = pps1.tile([128, CHUNK_COLS], F32, tag="psnd")
                        for u in range(U):
                            sg = sgp.tile([I, CHUNK_COLS], F16, tag="sg")
                            nc.scalar.activation(
                                sg[:], xc[:], AF.Sigmoid,
                                bias=c["sens_bias"][:, u:u + 1],
                                scale=c["sens_scale"][:, u:u + 1])
                            for n in range(CHUNK_COLS // 512):
                                nc.tensor.matmul(
                                    ps_nd[:, 512 * n:512 * n + 512],
                                    c["sens_lhsT"][:, 128 * u:128 * u + 128],
                                    sg[:, 512 * n:512 * n + 512],
                                    start=(u == 0), stop=(u == U - 1))
                        ev = pool1.tile([128, CHUNK_COLS], F16, tag="ev")
                        nc.vector.tensor_copy(ev[:], ps_nd[:])
                        nc.sync.dma_start(
                            d_scr[:, bass.ds(iv, CHUNK_COLS)], ev[:])

                    loop_i(0, n_chunks * CHUNK_COLS, CHUNK_COLS, p1_body)

              for _repp2 in range(rep_p2):
                # =================== phase 2 ===================
                with ExitStack() as pst:
                  spool = pst.enter_context(tc.tile_pool(name="p2state", bufs=1))
                  vstT = spool.tile([U + 1, B_CORE], F16, tag="vstT")
                  nc.vector.memset(vstT[0:U, :], 0.0)
                  nc.vector.memset(vstT[U:U + 1, :], 1.0)

                  with ExitStack() as p2:
                    pool2 = p2.enter_context(tc.tile_pool(name="p2", bufs=2))
                    sndp = p2.enter_context(tc.tile_pool(name="p2snd", bufs=3))
                    sigp = p2.enter_context(tc.tile_pool(name="p2sig", bufs=4))
                    ppsa = p2.enter_context(
                        tc.tile_pool(name="p2psa",
                                     bufs=2 if "nch2" in ablate else 4,
                                     space="PSUM"))
                    ppsr = p2.enter_context(
                        tc.tile_pool(name="p2psr", bufs=3, space="PSUM"))

                    def p2_body(iv):
                        snd_t = sndp.tile([128, B_CORE], F16, tag="snd")
                        if "nodma" in ablate:
                            nc.sync.dma_start(snd_t[:], d_scr[:, 0:B_CORE])
                        else:
                            nc.sync.dma_start(
                                snd_t[:], d_scr[:, bass.ds(iv, B_CORE)])
                        base = pool2.tile([128, B_CORE], F32, tag="base")
                        nc.gpsimd.tensor_scalar(
                            base[:], snd_t[:], c["CNCD_T"][:, 0:1], None,
                            op0=ALU.add)

                        for uf in range(UNFOLDS):
                            out_red = ppsr.tile([128, B_CORE], F32, tag="ored")
                            # base2 = cm_t*v + num-part of base, off-chain
                            cmtv = pool2.tile([U, B_CORE], F32, tag="cmtv")
                            nc.gpsimd.tensor_scalar(
                                cmtv[:], vstT[0:U, :], c["cmt_T"][:, 0:1],
                                None, op0=ALU.mult)
                            base2 = pool2.tile([U, B_CORE], F32, tag="base2")
                            nc.gpsimd.tensor_add(base2[:], cmtv[:],
                                                 base[0:U, :])
                            NCH = 2 if "nch2" in ablate else 4
                            NJ = 32 // NCH
                            for ch in range(NCH):    # chunks of NJ j each
                                psT = ppsa.tile([128, NJ * B_CORE], F32,
                                                tag="psT")
                                for jj in range(NJ):
                                    j = NJ * ch + jj
                                    nc.tensor.matmul(
                                        psT[:, 64 * jj:64 * jj + 64],
                                        c["S_arg"][:, 128 * j:128 * j + 128],
                                        vstT[:], start=True, stop=True)
                                sig = sigp.tile([128, NJ * B_CORE], F16,
                                                tag="sig")
                                nc.scalar.activation(sig[:], psT[:],
                                                     AF.Sigmoid)
                                if "nored" in ablate:
                                    continue
                                for jj in range(NJ):
                                    j = NJ * ch + jj
                                    nc.tensor.matmul(
                                        out_red[:],
                                        c["S_red"][:, 128 * j:128 * j + 128],
                                        sig[:, 64 * jj:64 * jj + 64],
                                        start=(j == 0), stop=(j == 31))
                            if "notail" in ablate or "nored" in ablate:
                                nc.gpsimd.tensor_copy(vstT[0:U, :], base2[:])
                                continue
                            totD = pool2.tile([U, B_CORE], F32, tag="totD")
                            nc.vector.tensor_add(totD[:], out_red[U:2 * U, :],
                                                 base[U:2 * U, :])
                            rec = pool2.tile([U, B_CORE], F32, tag="rec")
                            nc.vector.reciprocal(rec[:], totD[:])
                            totN = pool2.tile([U, B_CORE], F32, tag="totN")
                            nc.vector.tensor_add(totN[:], out_red[0:U, :],
                                                 base2[:])
                            v_new = pool2.tile([U, B_CORE], F32, tag="vnew")
                            nc.vector.tensor_mul(v_new[:], totN[:], rec[:])
                            nc.gpsimd.tensor_copy(vstT[0:U, :], v_new[:])

                    loop_i(0, T * B_CORE, B_CORE, p2_body)

                  # =================== head ===================
                  with ExitStack() as ph:
                    poolh = ph.enter_context(tc.tile_pool(name="ph", bufs=1))
                    ppsh = ph.enter_context(
                        tc.tile_pool(name="phps", bufs=1, space="PSUM"))
                    _emit_head(nc, tc, c, poolh, ppsh, vstT, d_out, mybir)

    nc.finalize()
    return nc


def _make_xT(x_core, T):
    """(B_CORE, T, I) -> fp16 [I, T*B_CORE] with col = t*64 + b."""
    xt = np.ascontiguousarray(x_core.transpose(2, 1, 0))  # (I, T, B)
    return xt.reshape(I, T * B_CORE).astype(np.float16)


_PROGRAM_CACHE = {}


def kernel(**inputs):
    import sys
    if '/opt/trn_rl_repo' not in sys.path:
        sys.path.insert(0, '/opt/trn_rl_repo')
    from concourse.bass_utils import run_bass_kernel_spmd

    x = np.asarray(inputs["x"], np.float32)
    B, T = x.shape[0], x.shape[1]
    consts = _host_consts(**{k: np.asarray(v) for k, v in inputs.items()
                             if k != "x"})

    if T not in _PROGRAM_CACHE:
        _PROGRAM_CACHE[T] = _build_program(T)
    nc = _PROGRAM_CACHE[T]

    in_maps = []
    for g in range(N_CORES):
        m = dict(consts)
        m["xT"] = _make_xT(x[g * B_CORE:(g + 1) * B_CORE], T)
        in_maps.append(m)
    res = run_bass_kernel_spmd(nc, in_maps, list(range(N_CORES)))
    return np.concatenate([res.results[g]["out"] for g in range(N_CORES)],
                          axis=0)


# revision 33
# speedup vs baseline: 7.2429x; 1.0693x over previous
"""Trainium2 Bass kernel for the EnhancedNavigationLTC model (v2, PE-centric).

Pure data parallel over batch: 512 rows sharded 8 ways (64 per core); all
parameters replicated.  Per core:

  phase 1 (parallel over B*T): sensory reduction.  Per 16-step chunk, 64
      ACT sigmoids (scale/bias folded per unit) in fp16 feed accumulating
      PE matmuls whose stationary rows are (nd, v)-major, producing
      snd[(nd, v), (t, b)] written to DRAM scratch in fp16.

  phase 2 (sequential, T steps x 4 unfolds): everything PE-centric in
      "transposed" space (units on partitions, batch in free):
        argT_j[(p2, v), b] = esig[u,v]*vst[b,u] - esigmu[u,v],  u = 2j+p2,
      built by 32 selector matmuls (stationary S_arg[j] [65,128], moving
      vstT [65,64] fp16); sigmoid runs on ACT (PSUM->SBUF fp16); the two
      weighted reductions over u are 32 accumulating matmuls (stationary
      S_red[j] [128,128] = wp / wp*erev diagonal-in-v) into a single
      out_red[(nd, v), b] PSUM tile.  The ODE update runs on [64,64] tiles
      where cm_t/gleak are per-partition scalars; the state feeds back with
      one fp16 copy (no transposes anywhere in the loop).

  head: output affine + LayerNorm + FC, on-device (as baseline).
"""

import numpy as np

U = 64
I = 128
O = 15
UNFOLDS = 4
EPS = 1e-8
LN_EPS = 1e-5
B_FULL, T_FULL = 512, 512
N_CORES = 8
B_CORE = B_FULL // N_CORES          # 64
TS_CHUNK = 16                       # timesteps per phase-1 chunk
CHUNK_COLS = TS_CHUNK * B_CORE      # 1024


def _softplus(x):
    return np.log1p(np.exp(-np.abs(x))) + np.maximum(x, 0.0)


def _host_consts(input_w, input_b, sensory_w, sensory_mu, sensory_sigma,
                 sensory_erev, w, mu, sigma, erev, gleak, vleak, cm,
                 output_w, output_b, ln_w, ln_b, fc_w, fc_b):
    """All parameter-derived device constants as numpy arrays."""
    f32, f16 = np.float32, np.float16
    wp = _softplus(np.asarray(w, f32))                  # (U, U)  [u, v]
    swp = _softplus(np.asarray(sensory_w, f32))         # (I, U)
    gleak_p = _softplus(np.asarray(gleak, f32))         # (U,)
    cm_t = (_softplus(np.asarray(cm, f32)) * UNFOLDS)   # (U,)

    # ---- phase 1 (sensory) ----
    se = np.asarray(sensory_erev, f32)                  # +-1  (I, U)
    ss = np.asarray(sensory_sigma, f32)
    iw = np.asarray(input_w, f32)
    ib = np.asarray(input_b, f32)
    smu = np.asarray(sensory_mu, f32)
    sens_scale = (se * ss * iw[:, None]).astype(f32)                # (I, U)
    sens_bias = (se * ss * (ib[:, None] - smu)).astype(f32)         # (I, U)
    Ks = (swp * (se < 0)).sum(axis=0).astype(f32)       # (U,)

    # stationary per u: [I, 128] with col m=(nd,v): nd=0 -> swp, nd=1 -> swp*se
    sens_lhsT = np.zeros((I, U, 2 * U), f32)
    for u in range(U):
        sens_lhsT[:, u, u] = swp[:, u]
        sens_lhsT[:, u, U + u] = swp[:, u] * se[:, u]
    sens_lhsT = sens_lhsT.reshape(I, U * 2 * U).astype(f16)   # (128, 8192)

    # ---- phase 2 (recurrent) ----
    ee = np.asarray(erev, f32)                          # +-1  (U, U) [u, v]
    esig = ee * np.asarray(sigma, f32)
    esigmu = esig * np.asarray(mu, f32)
    K = (wp * (ee < 0)).sum(axis=0).astype(f32)         # (V,)

    # S_arg[j]: [65, 128], col m = (p2, v): argT = S_arg[j].T @ [vstT; ones]
    S_arg = np.zeros((32, U + 1, 128), f32)
    for j in range(32):
        for p2 in range(2):
            u = 2 * j + p2
            S_arg[j, u, 64 * p2:64 * p2 + 64] = esig[u, :]
            S_arg[j, U, 64 * p2:64 * p2 + 64] = -esigmu[u, :]
    S_arg = S_arg.transpose(1, 0, 2).reshape(U + 1, 32 * 128).astype(f16)

    # S_red[j]: [128, 128], row k=(p2, v'), col m=(nd, v): diag in v
    S_red = np.zeros((32, 128, 128), f32)
    for j in range(32):
        for p2 in range(2):
            u = 2 * j + p2
            for v in range(U):
                S_red[j, 64 * p2 + v, v] = wp[u, v]
                S_red[j, 64 * p2 + v, U + v] = wp[u, v] * ee[u, v]
    S_red = S_red.transpose(1, 0, 2).reshape(128, 32 * 128).astype(f16)

    glvl = gleak_p * np.asarray(vleak, f32)
    CN = -K + glvl - Ks                                 # add to num rows
    CD = cm_t + gleak_p + K + Ks + EPS                  # add to den rows
    CNCD_T = np.concatenate([CN, CD])[:, None].astype(f32)   # [128, 1]
    cmt_T = cm_t[:, None].astype(f32)                        # [64, 1]

    # ---- head ----
    tile_b = lambda vec: np.tile(np.asarray(vec, f32)[None, :], (B_CORE, 1))
    return {
        "sens_scale": sens_scale, "sens_bias": sens_bias,
        "sens_lhsT": sens_lhsT,
        "S_arg": S_arg, "S_red": S_red,
        "CNCD_T": CNCD_T, "cmt_T": cmt_T,
        "ident": np.eye(64, dtype=f16),
        "identf": np.eye(64, dtype=f32),
        "ow_l": tile_b(output_w), "ob_l": tile_b(output_b),
        "lng_l": tile_b(ln_w), "lnb_l": tile_b(ln_b),
        "fcwT": np.asarray(fc_w, f32).T.copy().astype(f32),  # (U, O)
        "fcb_l": tile_b(fc_b),                               # (B_CORE, O)
    }


def _emit_head(nc, tc, c, poolh, ppsh, vstT, d_out, mybir):
    F32 = mybir.dt.float32
    ALU = mybir.AluOpType
    AX = mybir.AxisListType
    vst32 = poolh.tile([U, B_CORE], F32, tag="vst32")
    nc.scalar.copy(vst32[:], vstT[0:U, :])
    hp = ppsh.tile([B_CORE, U], F32, tag="hp")
    nc.tensor.transpose(hp[:], vst32[:], c["identf"][:])
    h = poolh.tile([B_CORE, U], F32, tag="h")
    nc.vector.tensor_mul(h[:], hp[:], c["ow_l"][:])
    nc.vector.tensor_add(h[:], h[:], c["ob_l"][:])
    mean = poolh.tile([B_CORE, 1], F32, tag="mean")
    nc.vector.tensor_reduce(mean[:], h[:], axis=AX.X, op=ALU.add)
    nc.vector.tensor_scalar_mul(mean[:], mean[:], 1.0 / U)
    xc2 = poolh.tile([B_CORE, U], F32, tag="xc2")
    nc.vector.tensor_scalar(xc2[:], h[:], mean[:], None, op0=ALU.subtract)
    sq = poolh.tile([B_CORE, U], F32, tag="sq")
    nc.vector.tensor_mul(sq[:], xc2[:], xc2[:])
    var = poolh.tile([B_CORE, 1], F32, tag="var")
    nc.vector.tensor_reduce(var[:], sq[:], axis=AX.X, op=ALU.add)
    nc.vector.tensor_scalar(
        var[:], var[:], 1.0 / U, LN_EPS, op0=ALU.mult, op1=ALU.add)
    sd = poolh.tile([B_CORE, 1], F32, tag="sd")
    nc.scalar.sqrt(sd[:], var[:])
    rstd = poolh.tile([B_CORE, 1], F32, tag="rstd")
    nc.vector.reciprocal(rstd[:], sd[:])
    nc.vector.tensor_scalar(xc2[:], xc2[:], rstd[:], None, op0=ALU.mult)
    nc.vector.tensor_mul(xc2[:], xc2[:], c["lng_l"][:])
    nc.vector.tensor_add(xc2[:], xc2[:], c["lnb_l"][:])
    hTp = ppsh.tile([U, B_CORE], F32, tag="hTp")
    nc.tensor.transpose(hTp[:], xc2[:], c["identf"][:])
    hT = poolh.tile([U, B_CORE], F32, tag="hT")
    nc.scalar.copy(hT[:], hTp[:])
    ps_fc = ppsh.tile([B_CORE, O], F32, tag="psfc")
    nc.tensor.matmul(ps_fc[:], hT[:], c["fcwT"][:], start=True, stop=True)
    res = poolh.tile([B_CORE, O], F32, tag="res")
    nc.vector.tensor_add(res[:], ps_fc[:], c["fcb_l"][:])
    nc.sync.dma_start(d_out[:], res[:])


def _build_program(T, repeats=1, unroll=False, rep_p1=1, rep_p2=1,
                   ablate=(), interleave=False):
    """Build the per-core SPMD Bass program for T timesteps.

    repeats>1 re-runs the whole computation (timing by differencing).
    unroll=True replaces For_i hardware loops with Python unrolling (for
    TimelineSim, which cannot resolve register-mode branches).
    """
    import concourse.bass as bass
    import concourse.tile as tile
    from concourse import bacc, mybir
    from contextlib import ExitStack

    F32 = mybir.dt.float32
    F16 = mybir.dt.float16
    AF = mybir.ActivationFunctionType
    ALU = mybir.AluOpType
    AX = mybir.AxisListType

    n_chunks = T // TS_CHUNK
    nc = bacc.Bacc("TRN2", target_bir_lowering=False, debug=False)

    d_xT = nc.dram_tensor("xT", [I, T * B_CORE], F16, kind="ExternalInput")
    cshape = {
        "sens_scale": ([I, U], F32), "sens_bias": ([I, U], F32),
        "sens_lhsT": ([I, U * 2 * U], F16),
        "S_arg": ([U + 1, 32 * 128], F16), "S_red": ([128, 32 * 128], F16),
        "CNCD_T": ([128, 1], F32), "cmt_T": ([U, 1], F32),
        "ident": ([U, U], F16), "identf": ([U, U], F32),
        "ow_l": ([B_CORE, U], F32), "ob_l": ([B_CORE, U], F32),
        "lng_l": ([B_CORE, U], F32), "lnb_l": ([B_CORE, U], F32),
        "fcwT": ([U, O], F32), "fcb_l": ([B_CORE, O], F32),
    }
    d_c = {k: nc.dram_tensor(k, shp, dt, kind="ExternalInput")
           for k, (shp, dt) in cshape.items()}
    d_out = nc.dram_tensor("out", [B_CORE, O], F32, kind="ExternalOutput")
    # scratch [128=(nd,v), (T+2)*64] (padded for prefetch overrun)
    d_scr = nc.dram_tensor("scr", [128, (T + 2) * B_CORE], F16)

    with tile.TileContext(nc) as tc:
        with ExitStack() as ctx:
            cpool = ctx.enter_context(tc.tile_pool(name="consts", bufs=1))
            c = {}
            for k, (shp, dt) in cshape.items():
                c[k] = cpool.tile(shp, dt, tag=k, name=k)
                nc.sync.dma_start(c[k][:], d_c[k][:])

            def loop_i(lo, hi, step, body):
                if unroll:
                    for i in range(lo, hi, step):
                        body(i)
                else:
                    with tc.For_i(lo, hi, step) as iv:
                        body(iv)

            for _rep in range(repeats):
              if interleave:
                with ExitStack() as pst_i:
                  spool = pst_i.enter_context(
                      tc.tile_pool(name="p2state", bufs=1))
                  with ExitStack() as pall:
                    pool1 = pall.enter_context(tc.tile_pool(name="p1", bufs=2))
                    sgp = pall.enter_context(tc.tile_pool(name="p1sg", bufs=3))
                    pps1 = pall.enter_context(
                        tc.tile_pool(name="p1ps", bufs=1, space="PSUM"))
                    pool2 = pall.enter_context(tc.tile_pool(name="p2", bufs=2))
                    sndp = pall.enter_context(
                        tc.tile_pool(name="p2snd", bufs=3))
                    sigp = pall.enter_context(
                        tc.tile_pool(name="p2sig", bufs=3))
                    ppsa = pall.enter_context(
                        tc.tile_pool(name="p2psa", bufs=3, space="PSUM"))
                    ppsr = pall.enter_context(
                        tc.tile_pool(name="p2psr", bufs=2, space="PSUM"))

                    vstT = spool.tile([U + 1, B_CORE], F16, tag="vstT")
                    nc.vector.memset(vstT[0:U, :], 0.0)
                    nc.vector.memset(vstT[U:U + 1, :], 1.0)

                    def p1_sig_mm(xc, ps_nd, u):
                        sg = sgp.tile([I, CHUNK_COLS], F16, tag="sg")
                        nc.scalar.activation(
                            sg[:], xc[:], AF.Sigmoid,
                            bias=c["sens_bias"][:, u:u + 1],
                            scale=c["sens_scale"][:, u:u + 1])
                        for n in range(CHUNK_COLS // 512):
                            nc.tensor.matmul(
                                ps_nd[:, 512 * n:512 * n + 512],
                                c["sens_lhsT"][:, 128 * u:128 * u + 128],
                                sg[:, 512 * n:512 * n + 512],
                                start=(u == 0), stop=(u == U - 1))

                    def p1_flush(ps_nd, col0):
                        ev = pool1.tile([128, CHUNK_COLS], F16, tag="ev")
                        nc.vector.tensor_copy(ev[:], ps_nd[:])
                        nc.sync.dma_start(
                            d_scr[:, bass.ds(col0, CHUNK_COLS)], ev[:])

                    def p2_step(iv, s):
                        snd_t = sndp.tile([128, B_CORE], F16, tag="snd")
                        nc.sync.dma_start(
                            snd_t[:],
                            d_scr[:, bass.ds(iv + B_CORE * s, B_CORE)])
                        base = pool2.tile([128, B_CORE], F32, tag="base")
                        nc.gpsimd.tensor_scalar(
                            base[:], snd_t[:], c["CNCD_T"][:, 0:1], None,
                            op0=ALU.add)
                        for uf in range(UNFOLDS):
                            out_red = ppsr.tile([128, B_CORE], F32,
                                                tag="ored")
                            cmtv = pool2.tile([U, B_CORE], F32, tag="cmtv")
                            nc.gpsimd.tensor_scalar(
                                cmtv[:], vstT[0:U, :], c["cmt_T"][:, 0:1],
                                None, op0=ALU.mult)
                            for ch in range(4):
                                psT = ppsa.tile([128, 8 * B_CORE], F32,
                                                tag="psT")
                                for jj in range(8):
                                    j = 8 * ch + jj
                                    nc.tensor.matmul(
                                        psT[:, 64 * jj:64 * jj + 64],
                                        c["S_arg"][:, 128 * j:128 * j + 128],
                                        vstT[:], start=True, stop=True)
                                sig = sigp.tile([128, 8 * B_CORE], F16,
                                                tag="sig")
                                nc.scalar.activation(sig[:], psT[:],
                                                     AF.Sigmoid)
                                for jj in range(8):
                                    j = 8 * ch + jj
                                    nc.tensor.matmul(
                                        out_red[:],
                                        c["S_red"][:, 128 * j:128 * j + 128],
                                        sig[:, 64 * jj:64 * jj + 64],
                                        start=(j == 0), stop=(j == 31))
                            tot = pool2.tile([128, B_CORE], F32, tag="tot")
                            nc.vector.tensor_add(tot[:], out_red[:], base[:])
                            rec = pool2.tile([U, B_CORE], F32, tag="rec")
                            nc.vector.reciprocal(rec[:], tot[U:2 * U, :])
                            num2 = pool2.tile([U, B_CORE], F32, tag="num2")
                            nc.vector.tensor_add(num2[:], tot[0:U, :],
                                                 cmtv[:])
                            v_new = pool2.tile([U, B_CORE], F32, tag="vnew")
                            nc.vector.tensor_mul(v_new[:], num2[:], rec[:])
                            nc.gpsimd.tensor_copy(vstT[0:U, :], v_new[:])

                    # prologue: full phase-1 chunk 0
                    xc0 = pool1.tile([I, CHUNK_COLS], F16, tag="xc")
                    nc.sync.dma_start(xc0[:], d_xT[:, 0:CHUNK_COLS])
                    ps0 = pps1.tile([128, CHUNK_COLS], F32, tag="psnd")
                    for u in range(U):
                        p1_sig_mm(xc0, ps0, u)
                    p1_flush(ps0, 0)

                    def pair_body(iv, last=False):
                        if not last:
                            xc = pool1.tile([I, CHUNK_COLS], F16, tag="xc")
                            nc.sync.dma_start(
                                xc[:],
                                d_xT[:, bass.ds(iv + CHUNK_COLS,
                                                CHUNK_COLS)])
                            ps_nd = pps1.tile([128, CHUNK_COLS], F32,
                                              tag="psnd")
                        for s in range(TS_CHUNK):
                            p2_step(iv, s)
                            if not last:
                                for q in range(4):
                                    p1_sig_mm(xc, ps_nd, 4 * s + q)
                        if not last:
                            p1_flush(ps_nd, iv + CHUNK_COLS)

                    loop_i(0, (n_chunks - 1) * CHUNK_COLS, CHUNK_COLS,
                           pair_body)
                    pair_body((n_chunks - 1) * CHUNK_COLS, last=True)

                  # ---- head (p1/p2 pools released) ----
                  with ExitStack() as ph:
                    poolh = ph.enter_context(
                        tc.tile_pool(name="ph", bufs=1))
                    ppsh = ph.enter_context(
                        tc.tile_pool(name="phps", bufs=1, space="PSUM"))
                    _emit_head(nc, tc, c, poolh, ppsh, vstT, d_out,
                               mybir)
                continue
              for _repp1 in range(rep_p1):
                # ============ phase 1 ============
                with ExitStack() as p1:
                    pool1 = p1.enter_context(tc.tile_pool(name="p1", bufs=2))
                    sgp = p1.enter_context(tc.tile_pool(name="p1sg", bufs=3))
                    pps1 = p1.enter_context(
                        tc.tile_pool(name="p1ps", bufs=2, space="PSUM"))

                    def p1_body(iv):
                        xc = pool1.tile([I, CHUNK_COLS], F16, tag="xc")
                        nc.sync.dma_start(xc[:], d_xT[:, bass.ds(iv, CHUNK_COLS)])
                        ps_nd = pps1.tile([128, CHUNK_COLS], F32, tag="psnd")
                        for u in range(U):
                            sg = sgp.tile([I, CHUNK_COLS], F16, tag="sg")
                            nc.scalar.activation(
                                sg[:], xc[:], AF.Sigmoid,
                                bias=c["sens_bias"][:, u:u + 1],
                                scale=c["sens_scale"][:, u:u + 1])
                            for n in range(CHUNK_COLS // 512):
                                nc.tensor.matmul(
                                    ps_nd[:, 512 * n:512 * n + 512],
                                    c["sens_lhsT"][:, 128 * u:128 * u + 128],
                                    sg[:, 512 * n:512 * n + 512],
                                    start=(u == 0), stop=(u == U - 1))
                        ev = pool1.tile([128, CHUNK_COLS], F16, tag="ev")
                        nc.vector.tensor_copy(ev[:], ps_nd[:])
                        nc.sync.dma_start(
                            d_scr[:, bass.ds(iv, CHUNK_COLS)], ev[:])

                    loop_i(0, n_chunks * CHUNK_COLS, CHUNK_COLS, p1_body)

              for _repp2 in range(rep_p2):
                # =================== phase 2 ===================
                with ExitStack() as pst:
                  spool = pst.enter_context(tc.tile_pool(name="p2state", bufs=1))
                  vstT = spool.tile([U + 1, B_CORE], F16, tag="vstT")
                  nc.vector.memset(vstT[0:U, :], 0.0)
                  nc.vector.memset(vstT[U:U + 1, :], 1.0)

                  with ExitStack() as p2:
                    pool2 = p2.enter_context(tc.tile_pool(name="p2", bufs=2))
                    sndp = p2.enter_context(tc.tile_pool(name="p2snd", bufs=3))
                    sigp = p2.enter_context(tc.tile_pool(name="p2sig", bufs=4))
                    ppsa = p2.enter_context(
                        tc.tile_pool(name="p2psa",
                                     bufs=2 if "nch2" in ablate else 4,
                                     space="PSUM"))
                    ppsr = p2.enter_context(
                        tc.tile_pool(name="p2psr", bufs=3, space="PSUM"))

                    def p2_body(iv):
                        snd_t = sndp.tile([128, B_CORE], F16, tag="snd")
                        if "nodma" in ablate:
                            nc.sync.dma_start(snd_t[:], d_scr[:, 0:B_CORE])
                        else:
                            nc.sync.dma_start(
                                snd_t[:], d_scr[:, bass.ds(iv, B_CORE)])
                        base = pool2.tile([128, B_CORE], F32, tag="base")
                        nc.gpsimd.tensor_scalar(
                            base[:], snd_t[:], c["CNCD_T"][:, 0:1], None,
                            op0=ALU.add)

                        for uf in range(UNFOLDS):
                            out_red = ppsr.tile([128, B_CORE], F32, tag="ored")
                            # base2 = cm_t*v + num-part of base, off-chain
                            cmtv = pool2.tile([U, B_CORE], F32, tag="cmtv")
                            nc.gpsimd.tensor_scalar(
                                cmtv[:], vstT[0:U, :], c["cmt_T"][:, 0:1],
                                None, op0=ALU.mult)
                            base2 = pool2.tile([U, B_CORE], F32, tag="base2")
                            nc.gpsimd.tensor_add(base2[:], cmtv[:],
                                                 base[0:U, :])
                            NCH = 2 if "nch2" in ablate else 4
                            NJ = 32 // NCH
                            for ch in range(NCH):    # chunks of NJ j each
                                psT = ppsa.tile([128, NJ * B_CORE], F32,
                                                tag="psT")
                                for jj in range(NJ):
                                    j = NJ * ch + jj
                                    nc.tensor.matmul(
                                        psT[:, 64 * jj:64 * jj + 64],
                                        c["S_arg"][:, 128 * j:128 * j + 128],
                                        vstT[:], start=True, stop=True)
                                sig = sigp.tile([128, NJ * B_CORE], F16,
                                                tag="sig")
                                nc.scalar.activation(sig[:], psT[:],
                                                     AF.Sigmoid)
                                if "nored" in ablate:
                                    continue
                                for jj in range(NJ):
                                    j = NJ * ch + jj
                                    nc.tensor.matmul(
                                        out_red[:],
                                        c["S_red"][:, 128 * j:128 * j + 128],
                                        sig[:, 64 * jj:64 * jj + 64],
                                        start=(j == 0), stop=(j == 31))
                            if "notail" in ablate or "nored" in ablate:
                                nc.gpsimd.tensor_copy(vstT[0:U, :], base2[:])
                                continue
                            totD = pool2.tile([U, B_CORE], F32, tag="totD")
                            nc.vector.tensor_add(totD[:], out_red[U:2 * U, :],
                                                 base[U:2 * U, :])
                            rec = pool2.tile([U, B_CORE], F32, tag="rec")
                            nc.vector.reciprocal(rec[:], totD[:])
                            totN = pool2.tile([U, B_CORE], F32, tag="totN")
                            nc.vector.tensor_add(totN[:], out_red[0:U, :],
                                                 base2[:])
                            # write the fp16 state directly: drops the
                            # Pool copy (one chain hop) per unfold
                            nc.vector.tensor_mul(vstT[0:U, :], totN[:],
                                                 rec[:])

                    loop_i(0, T * B_CORE, B_CORE, p2_body)

                  # =================== head ===================
                  with ExitStack() as ph:
                    poolh = ph.enter_context(tc.tile_pool(name="ph", bufs=1))
                    ppsh = ph.enter_context(
                        tc.tile_pool(name="phps", bufs=1, space="PSUM"))
                    _emit_head(nc, tc, c, poolh, ppsh, vstT, d_out, mybir)

    nc.finalize()
    return nc


def _make_xT(x_core, T):
    """(B_CORE, T, I) -> fp16 [I, T*B_CORE] with col = t*64 + b."""
    xt = np.ascontiguousarray(x_core.transpose(2, 1, 0))  # (I, T, B)
    return xt.reshape(I, T * B_CORE).astype(np.float16)


_PROGRAM_CACHE = {}


def kernel(**inputs):
    import sys
    if '/opt/trn_rl_repo' not in sys.path:
        sys.path.insert(0, '/opt/trn_rl_repo')
    from concourse.bass_utils import run_bass_kernel_spmd

    x = np.asarray(inputs["x"], np.float32)
    B, T = x.shape[0], x.shape[1]
    consts = _host_consts(**{k: np.asarray(v) for k, v in inputs.items()
                             if k != "x"})

    if T not in _PROGRAM_CACHE:
        _PROGRAM_CACHE[T] = _build_program(T)
    nc = _PROGRAM_CACHE[T]

    in_maps = []
    for g in range(N_CORES):
        m = dict(consts)
        m["xT"] = _make_xT(x[g * B_CORE:(g + 1) * B_CORE], T)
        in_maps.append(m)
    res = run_bass_kernel_spmd(nc, in_maps, list(range(N_CORES)))
    return np.concatenate([res.results[g]["out"] for g in range(N_CORES)],
                          axis=0)
